# revision 8
# baseline (speedup 1.0000x reference)
"""Trainium2 Bass kernel: causal self-attention with sigmoid attention and
Bernoulli straight-through sampling (nn_CausalSelfAttention_57002805953253).

Key facts this implementation exploits:

* In the forward pass, the straight-through estimator makes the attention
  matrix numerically EQUAL to the Bernoulli samples (att + (samp - att) ==
  samp exactly in fp32).  Therefore att_var == 0 exactly, att_sum is the
  per-row count of successes, and y is computed from the 0/1 samples.
* The Bernoulli draw is `u < sigmoid(scores)` with u = uniform(key, shape)
  for a FIXED key (input independent).  Since sigmoid is monotonic,
  `u < sigmoid(s)  <=>  s > logit(u)`, so the device kernel only needs the
  raw scores and a precomputed threshold tensor L = logit(u).  The causal
  mask is baked into L as +inf (s > +inf is always false).
* Sharding: core c handles batch b = c//2 and heads hg*8..hg*8+8 (hg=c%2).
  The final projection is computed per-core against the head-slice of
  w_proj; the host adds the two partial results per batch.

Precision strategy: scores (qkv q/k + q@k^T) run as float32r (fp32 data,
FP22 multiply, 2 cycles/row on this silicon) because sample flips scale
with score error.  The value path (v, samples @ v, projection) runs bf16
(1 cycle/row): samples are exactly representable and the value-path error
is dominated by the sample flips anyway.

Layout strategy: scores are computed transposed (S^T tiles [k,q]) so the
0/1 samples feed the y matmul with no transposes; a ones column appended
to v yields att_sum for free in the same matmul.  Even/odd heads' K=64
score matmuls use PE row-groups 0-63 / 64-127 and are emitted adjacently
so they execute concurrently.  qkv for head-pair j+1 is interleaved with
attention for pair j to keep the PE dense (HAM stays un-throttled), and
the big logit-threshold stream is issued round-robin across the three DMA
issue paths (sync HWDGE, scalar HWDGE, gpsimd SWDGE).
"""

import os
import sys
from contextlib import ExitStack

import numpy as np

for _p in ("/opt/trn_rl_repo", "/root/.axon_site/_ro/trn_rl_repo"):
    if os.path.isdir(_p) and _p not in sys.path:
        sys.path.append(_p)

import ml_dtypes  # noqa: E402
import concourse.bass as bass  # noqa: E402,F401
import concourse.tile as tile  # noqa: E402
from concourse import bacc, mybir  # noqa: E402
from concourse.bass_utils import run_bass_kernel_spmd  # noqa: E402

B, T, C, H = 4, 1024, 1024, 16
HS = C // H            # 64 head size
NCORES = 8
HPC = H // 2           # 8 local heads per core
F32 = mybir.dt.float32
F32R = mybir.dt.float32r
BF16 = mybir.dt.bfloat16


def _lt_blocks():
    """Causal block schedule: [(qc, kt, qs, n, off)] + packed stride."""
    blocks = []
    off = 0
    for qc in range(2):
        for kt in range(4 if qc == 0 else 8):
            qs = max(kt * 128, qc * 512)
            n = qc * 512 + 512 - qs
            blocks.append((qc, kt, qs, n, off))
            off += 128 * n
    return blocks, off


LT_BLOCKS, LT_STRIDE = _lt_blocks()   # stride = 4608*128 = 589824 fp32/head

_CACHE = {}


def build_nc():
    """Build the SPMD Bass program (identical on all 8 cores)."""
    nc = bacc.Bacc("TRN2", target_bir_lowering=False, debug=False)

    xt_d = nc.dram_tensor("xt", [C, T], F32R, kind="ExternalInput")
    wqt_d = nc.dram_tensor("wqt", [C, HPC * HS], F32R, kind="ExternalInput")
    wkt_d = nc.dram_tensor("wkt", [C, HPC * HS], F32R, kind="ExternalInput")
    wvt_d = nc.dram_tensor("wvt", [C, HPC * HS], BF16, kind="ExternalInput")
    wpt_d = nc.dram_tensor("wpt", [HPC * HS, C], BF16, kind="ExternalInput")
    ltp_d = nc.dram_tensor("ltp", [HPC, LT_STRIDE], F32, kind="ExternalInput")
    yp_d = nc.dram_tensor("yp", [T, C], F32, kind="ExternalOutput")
    asum_d = nc.dram_tensor("asum", [HPC, T], F32, kind="ExternalOutput")

    # round-robin DMA issue engines
    def dma_engines(nc):
        return [nc.sync, nc.scalar, nc.gpsimd]

    with tile.TileContext(nc) as tc, ExitStack() as ctx:
        persist = ctx.enter_context(tc.tile_pool(name="persist", bufs=1))
        ltpool = ctx.enter_context(tc.tile_pool(name="ltpool", bufs=6))
        smpool = ctx.enter_context(tc.tile_pool(name="smpool", bufs=6))
        opool = ctx.enter_context(tc.tile_pool(name="opool", bufs=2))
        astp = ctx.enter_context(tc.tile_pool(name="astp", bufs=2))
        psmm = ctx.enter_context(tc.tile_pool(name="psmm", bufs=4, space="PSUM"))
        psy = ctx.enter_context(tc.tile_pool(name="psy", bufs=4, space="PSUM"))

        engs = dma_engines(nc)
        rr = [0]

        def dma(dst, src):
            e = engs[rr[0] % 3]
            rr[0] += 1
            e.dma_start(dst, src)

        # ---- load x^T and weights -------------------------------------
        # x^T as bf16 too (for the bf16 v matmul) -- converted on device.
        xt_sb = []
        for i in range(8):
            t_ = persist.tile([128, T], F32R, tag=f"xt{i}", name=f"xt{i}")
            dma(t_[:, :], xt_d[i * 128:(i + 1) * 128, :])
            xt_sb.append(t_)
        xtb_sb = []
        for i in range(8):
            t_ = persist.tile([128, T], BF16, tag=f"xtb{i}", name=f"xtb{i}")
            nc.scalar.copy(t_[:, :], xt_sb[i][:, :])
            xtb_sb.append(t_)

        wsb = {}
        for name, dram, dt_ in (("wqt", wqt_d, F32R), ("wkt", wkt_d, F32R),
                                ("wvt", wvt_d, BF16)):
            tiles = []
            for i in range(8):
                t_ = persist.tile([128, HPC * HS], dt_,
                                  tag=f"{name}{i}", name=f"{name}{i}")
                dma(t_[:, :], dram[i * 128:(i + 1) * 128, :])
                tiles.append(t_)
            wsb[name] = tiles

        wpt_sb = []
        for j in range(4):
            t_ = persist.tile([128, C], BF16, tag=f"wpt{j}", name=f"wpt{j}")
            dma(t_[:, :], wpt_d[j * 128:(j + 1) * 128, :])
            wpt_sb.append(t_)

        # ---- v first (bf16), augmented with a ones column per head ----
        vaug = [persist.tile([128, HPC * (HS + 1)], BF16,
                             tag=f"va{t}", name=f"va{t}")
                for t in range(8)]
        for tt in range(8):
            ps = psmm.tile([128, 512], F32, tag="psmm", name="ps_v")
            for kc in range(8):
                nc.tensor.matmul(
                    ps[:, :],
                    xtb_sb[kc][:, tt * 128:(tt + 1) * 128],
                    wsb["wvt"][kc][:, :],
                    start=(kc == 0), stop=(kc == 7),
                )
            nc.scalar.copy(
                vaug[tt][:, :].rearrange("p (h e) -> p h e", e=HS + 1)[:, :, 0:HS],
                ps[:, :].rearrange("p (h e) -> p h e", e=HS),
            )
            nc.vector.memset(
                vaug[tt][:, :].rearrange("p (h e) -> p h e", e=HS + 1)[:, :, HS:HS + 1],
                1.0,
            )

        # ---- q^T / k^T for one head-pair j ----------------------------
        qt_sb = [persist.tile([128, T], F32R, tag=f"qt{j}", name=f"qt{j}")
                 for j in range(4)]
        kt_sb = [persist.tile([128, T], F32R, tag=f"kt{j}", name=f"kt{j}")
                 for j in range(4)]

        def emit_qk(j):
            for dst, wname in ((qt_sb, "wqt"), (kt_sb, "wkt")):
                for half in range(2):
                    ps = psmm.tile([128, 512], F32, tag="psmm", name="ps_qk")
                    for kc in range(8):
                        nc.tensor.matmul(
                            ps[:, :],
                            wsb[wname][kc][:, j * 128:(j + 1) * 128],
                            xt_sb[kc][:, half * 512:(half + 1) * 512],
                            start=(kc == 0), stop=(kc == 7),
                        )
                    nc.scalar.copy(
                        dst[j][:, half * 512:(half + 1) * 512], ps[:, :])

        # ---- attention for one head-pair j (heads 2j even, 2j+1 odd) --
        yt_sb = [persist.tile([128, T], BF16, tag=f"yt{j}", name=f"yt{j}")
                 for j in range(4)]

        def emit_att(j):
            for qc in range(2):
                nblk = [blk for blk in LT_BLOCKS if blk[0] == qc]
                first_kt, last_kt = nblk[0][1], nblk[-1][1]
                yps = [psy.tile([HS + 1, 512], F32, tag="psy", name="psy_e"),
                       psy.tile([HS + 1, 512], F32, tag="psy", name="psy_o")]
                for (_, kt, qs, n, off) in nblk:
                    sps = []
                    for par in range(2):           # even/odd: PE row groups
                        po = 64 * par
                        sp = psmm.tile([128, n], F32, tag="psmm", name="ps_s")
                        nc.tensor.matmul(
                            sp[:, :],
                            kt_sb[j][po:po + 64, kt * 128:(kt + 1) * 128],
                            qt_sb[j][po:po + 64, qs:qs + n],
                            start=True, stop=True,
                        )
                        sps.append(sp)
                    o = qs - qc * 512
                    for par in range(2):
                        i = 2 * j + par            # local head
                        ltb = ltpool.tile([128, n], F32, tag="lt", name="lt_t")
                        dma(ltb[:, :],
                            ltp_d[i, off:off + 128 * n].rearrange(
                                "(p m) -> p m", p=128))
                        smp = smpool.tile([128, n], BF16, tag="smp",
                                          name="smp_t")
                        nc.vector.tensor_tensor(
                            smp[:, :], sps[par][:, :], ltb[:, :],
                            op=mybir.AluOpType.is_gt,
                        )
                        nc.tensor.matmul(
                            yps[par][:, o:o + n],
                            vaug[kt][:, i * (HS + 1):(i + 1) * (HS + 1)],
                            smp[:, :],
                            start=(kt == first_kt), stop=(kt == last_kt),
                        )
                for par in range(2):
                    i = 2 * j + par
                    po = 64 * par
                    nc.scalar.copy(
                        yt_sb[j][po:po + 64, qc * 512:(qc + 1) * 512],
                        yps[par][0:64, :])
                    ast = astp.tile([65, 512], F32, tag="ast", name="ast_t")
                    nc.scalar.copy(ast[64:65, :], yps[par][64:65, :])
                    dma(asum_d[i, qc * 512:(qc + 1) * 512], ast[64:65, :])

        # interleave: qkv(j+1) emitted right after attention(j) starts
        emit_qk(0)
        for j in range(4):
            if j < 3:
                # emit next pair's qkv adjacent so Tile fills PE gaps
                emit_att(j)
                emit_qk(j + 1)
            else:
                emit_att(j)

        # ---- partial projection y_heads @ wproj_slice^T (bf16) --------
        for tt in range(8):
            ot = opool.tile([128, C], F32, tag="ot", name="ot_t")
            for nch in range(2):
                ps = psmm.tile([128, 512], F32, tag="psmm", name="ps_p")
                for jj in range(4):
                    nc.tensor.matmul(
                        ps[:, :],
                        yt_sb[jj][:, tt * 128:(tt + 1) * 128],
                        wpt_sb[jj][:, nch * 512:(nch + 1) * 512],
                        start=(jj == 0), stop=(jj == 3),
                    )
                nc.scalar.copy(ot[:, nch * 512:(nch + 1) * 512], ps[:, :])
            dma(yp_d[tt * 128:(tt + 1) * 128, :], ot[:, :])

    nc.compile()   # bacc register allocation + finalize before serialization
    return nc


def _get_nc():
    if "nc" not in _CACHE:
        _CACHE["nc"] = build_nc()
    return _CACHE["nc"]


def _get_u():
    """Exactly reproduce the uniform field jax.random.bernoulli draws.

    The container pins jax_default_prng_impl='rbg', whose bitstream is
    backend-dependent — so this must run on the same default device the
    reference uses (do NOT pin to CPU here).
    """
    import jax
    import jax.numpy as jnp
    samp_key = jax.random.fold_in(jax.random.key(0), 42)
    u = jax.random.uniform(samp_key, (B, H, T, T), dtype=jnp.float32)
    return np.asarray(u)


def _prep_ltp():
    """Pack per-core causal logit-threshold blocks: [NCORES, HPC, LT_STRIDE]."""
    if "ltp" in _CACHE:
        return _CACHE["ltp"]
    u = _get_u()
    ltp = np.empty((NCORES, HPC, LT_STRIDE), np.float32)
    kk = np.arange(128)[:, None]
    for c in range(NCORES):
        b, hg = c // 2, c % 2
        for i in range(HPC):
            h = hg * HPC + i
            for (qc, kt, qs, n, off) in LT_BLOCKS:
                ub = u[b, h, qs:qs + n, kt * 128:(kt + 1) * 128].astype(np.float64)
                with np.errstate(divide="ignore"):
                    lt = np.log(ub) - np.log1p(-ub)
                ltb = np.ascontiguousarray(lt.T).astype(np.float32)  # [128, n]
                if qs == kt * 128:
                    ltb[kk > np.arange(n)[None, :]] = np.inf
                ltp[c, i, off:off + 128 * n] = ltb.reshape(-1)
    _CACHE["ltp"] = ltp
    return ltp


def make_in_maps(x, w_attn, w_proj):
    scale = np.float32(1.0 / np.sqrt(np.float32(HS)))   # 0.125, exact pow2
    ltp = _prep_ltp()
    in_maps = []
    for c in range(NCORES):
        b, hg = c // 2, c % 2
        r0 = hg * HPC * HS
        in_maps.append({
            "xt": np.ascontiguousarray(x[b].T),
            "wqt": np.ascontiguousarray((w_attn[r0:r0 + 512, :] * scale).T),
            "wkt": np.ascontiguousarray(w_attn[C + r0:C + r0 + 512, :].T),
            "wvt": np.ascontiguousarray(
                w_attn[2 * C + r0:2 * C + r0 + 512, :].T).astype(
                    ml_dtypes.bfloat16),
            "wpt": np.ascontiguousarray(
                w_proj[:, r0:r0 + 512].T).astype(ml_dtypes.bfloat16),
            "ltp": ltp[c],
        })
    return in_maps


def assemble(results):
    """Combine per-core outputs into full outputs."""
    y = np.zeros((B, T, C), np.float32)
    att_sum = np.zeros((B, H, T), np.float32)
    for c in range(NCORES):
        b, hg = c // 2, c % 2
        y[b] += results[c]["yp"]
        att_sum[b, hg * HPC:(hg + 1) * HPC, :] = results[c]["asum"]
    att_var = np.zeros((B, H, T), np.float32)
    return y, att_sum, att_var


def kernel(x, w_attn, w_proj, **run_kwargs):
    x = np.asarray(x, dtype=np.float32)
    w_attn = np.asarray(w_attn, dtype=np.float32)
    w_proj = np.asarray(w_proj, dtype=np.float32)
    nc = _get_nc()
    in_maps = make_in_maps(x, w_attn, w_proj)
    res = run_bass_kernel_spmd(nc, in_maps, core_ids=list(range(NCORES)),
                               **run_kwargs)
    out = assemble(res.results)
    _CACHE["last_result"] = res
    return out


# revision 9
# speedup vs baseline: 1.0302x; 1.0302x over previous
"""Trainium2 Bass kernel: causal self-attention with sigmoid attention and
Bernoulli straight-through sampling (nn_CausalSelfAttention_57002805953253).

Key facts this implementation exploits:

* In the forward pass, the straight-through estimator makes the attention
  matrix numerically EQUAL to the Bernoulli samples (att + (samp - att) ==
  samp exactly in fp32).  Therefore att_var == 0 exactly, att_sum is the
  per-row count of successes, and y is computed from the 0/1 samples.
* The Bernoulli draw is `u < sigmoid(scores)` with u = uniform(key, shape)
  for a FIXED key (input independent).  Since sigmoid is monotonic,
  `u < sigmoid(s)  <=>  s > logit(u)`, so the device kernel only needs the
  raw scores and a precomputed threshold tensor L = logit(u).  The causal
  mask is baked into L as +inf (s > +inf is always false).
* Sharding: core c handles batch b = c//2 and heads hg*8..hg*8+8 (hg=c%2).
  The final projection is computed per-core against the head-slice of
  w_proj; the host adds the two partial results per batch.

Precision strategy: scores (qkv q/k + q@k^T) run as float32r (fp32 data,
FP22 multiply, 2 cycles/row on this silicon) because sample flips scale
with score error.  The value path (v, samples @ v, projection) runs bf16
(1 cycle/row): samples are exactly representable and the value-path error
is dominated by the sample flips anyway.

Layout strategy: scores are computed transposed (S^T tiles [k,q]) so the
0/1 samples feed the y matmul with no transposes; a ones column appended
to v yields att_sum for free in the same matmul.  Even/odd heads' K=64
score matmuls use PE row-groups 0-63 / 64-127 and are emitted adjacently
so they execute concurrently.  qkv for head-pair j+1 is interleaved with
attention for pair j to keep the PE dense (HAM stays un-throttled), and
the big logit-threshold stream is issued round-robin across the three DMA
issue paths (sync HWDGE, scalar HWDGE, gpsimd SWDGE).
"""

import os
import sys
from contextlib import ExitStack

import numpy as np

for _p in ("/opt/trn_rl_repo", "/root/.axon_site/_ro/trn_rl_repo"):
    if os.path.isdir(_p) and _p not in sys.path:
        sys.path.append(_p)

import ml_dtypes  # noqa: E402
import concourse.bass as bass  # noqa: E402,F401
import concourse.tile as tile  # noqa: E402
from concourse import bacc, mybir  # noqa: E402
from concourse.bass_utils import run_bass_kernel_spmd  # noqa: E402

B, T, C, H = 4, 1024, 1024, 16
HS = C // H            # 64 head size
NCORES = 8
HPC = H // 2           # 8 local heads per core
F32 = mybir.dt.float32
F32R = mybir.dt.float32r
BF16 = mybir.dt.bfloat16


def _lt_blocks():
    """Causal block schedule: [(qc, kt, qs, n, off)] + packed stride."""
    blocks = []
    off = 0
    for qc in range(2):
        for kt in range(4 if qc == 0 else 8):
            qs = max(kt * 128, qc * 512)
            n = qc * 512 + 512 - qs
            blocks.append((qc, kt, qs, n, off))
            off += 128 * n
    return blocks, off


LT_BLOCKS, LT_STRIDE = _lt_blocks()   # stride = 4608*128 = 589824 fp32/head

_CACHE = {}


def build_nc():
    """Build the SPMD Bass program (identical on all 8 cores)."""
    nc = bacc.Bacc("TRN2", target_bir_lowering=False, debug=False)

    xt_d = nc.dram_tensor("xt", [C, T], F32R, kind="ExternalInput")
    wqt_d = nc.dram_tensor("wqt", [C, HPC * HS], F32R, kind="ExternalInput")
    wkt_d = nc.dram_tensor("wkt", [C, HPC * HS], F32R, kind="ExternalInput")
    wvt_d = nc.dram_tensor("wvt", [C, HPC * HS], BF16, kind="ExternalInput")
    wpt_d = nc.dram_tensor("wpt", [HPC * HS, C], BF16, kind="ExternalInput")
    ltp_d = nc.dram_tensor("ltp", [HPC, LT_STRIDE], F32, kind="ExternalInput")
    yp_d = nc.dram_tensor("yp", [T, C], F32, kind="ExternalOutput")
    asum_d = nc.dram_tensor("asum", [HPC, T], F32, kind="ExternalOutput")

    # round-robin DMA issue engines
    def dma_engines(nc):
        return [nc.sync, nc.scalar, nc.gpsimd]

    with tile.TileContext(nc) as tc, ExitStack() as ctx:
        persist = ctx.enter_context(tc.tile_pool(name="persist", bufs=1))
        ltpool = ctx.enter_context(tc.tile_pool(name="ltpool", bufs=12))
        smpool = ctx.enter_context(tc.tile_pool(name="smpool", bufs=10))
        opool = ctx.enter_context(tc.tile_pool(name="opool", bufs=2))
        astp = ctx.enter_context(tc.tile_pool(name="astp", bufs=2))
        psmm = ctx.enter_context(tc.tile_pool(name="psmm", bufs=4, space="PSUM"))
        psy = ctx.enter_context(tc.tile_pool(name="psy", bufs=4, space="PSUM"))

        engs = dma_engines(nc)
        rr = [0]

        def dma(dst, src):
            e = engs[rr[0] % 3]
            rr[0] += 1
            e.dma_start(dst, src)

        # ---- load x^T and weights (interleaved so k-tile 0 lands first)
        xt_sb, xtb_sb = [], []
        wsb = {"wqt": [], "wkt": [], "wvt": []}
        for i in range(8):
            for name, dram, dt_ in (("wvt", wvt_d, BF16), ("wqt", wqt_d, F32R),
                                    ("wkt", wkt_d, F32R)):
                t_ = persist.tile([128, HPC * HS], dt_,
                                  tag=f"{name}{i}", name=f"{name}{i}")
                dma(t_[:, :], dram[i * 128:(i + 1) * 128, :])
                wsb[name].append(t_)
            t_ = persist.tile([128, T], F32R, tag=f"xt{i}", name=f"xt{i}")
            dma(t_[:, :], xt_d[i * 128:(i + 1) * 128, :])
            xt_sb.append(t_)
            tb = persist.tile([128, T], BF16, tag=f"xtb{i}", name=f"xtb{i}")
            nc.scalar.copy(tb[:, :], t_[:, :])
            xtb_sb.append(tb)

        wpt_sb = []
        for j in range(4):
            t_ = persist.tile([128, C], BF16, tag=f"wpt{j}", name=f"wpt{j}")
            dma(t_[:, :], wpt_d[j * 128:(j + 1) * 128, :])
            wpt_sb.append(t_)

        # ---- v first (bf16), augmented with a ones column per head ----
        vaug = [persist.tile([128, HPC * (HS + 1)], BF16,
                             tag=f"va{t}", name=f"va{t}")
                for t in range(8)]
        for tt in range(8):
            ps = psmm.tile([128, 512], F32, tag="psmm", name="ps_v")
            for kc in range(8):
                nc.tensor.matmul(
                    ps[:, :],
                    xtb_sb[kc][:, tt * 128:(tt + 1) * 128],
                    wsb["wvt"][kc][:, :],
                    start=(kc == 0), stop=(kc == 7),
                )
            nc.scalar.copy(
                vaug[tt][:, :].rearrange("p (h e) -> p h e", e=HS + 1)[:, :, 0:HS],
                ps[:, :].rearrange("p (h e) -> p h e", e=HS),
            )
            nc.vector.memset(
                vaug[tt][:, :].rearrange("p (h e) -> p h e", e=HS + 1)[:, :, HS:HS + 1],
                1.0,
            )

        # ---- q^T / k^T for one head-pair j ----------------------------
        qt_sb = [persist.tile([128, T], F32R, tag=f"qt{j}", name=f"qt{j}")
                 for j in range(4)]
        kt_sb = [persist.tile([128, T], F32R, tag=f"kt{j}", name=f"kt{j}")
                 for j in range(4)]

        def emit_qk(j):
            for dst, wname in ((qt_sb, "wqt"), (kt_sb, "wkt")):
                for half in range(2):
                    ps = psmm.tile([128, 512], F32, tag="psmm", name="ps_qk")
                    for kc in range(8):
                        nc.tensor.matmul(
                            ps[:, :],
                            wsb[wname][kc][:, j * 128:(j + 1) * 128],
                            xt_sb[kc][:, half * 512:(half + 1) * 512],
                            start=(kc == 0), stop=(kc == 7),
                        )
                    nc.scalar.copy(
                        dst[j][:, half * 512:(half + 1) * 512], ps[:, :])

        # ---- attention for one head-pair j (heads 2j even, 2j+1 odd) --
        yt_sb = [persist.tile([128, T], BF16, tag=f"yt{j}", name=f"yt{j}")
                 for j in range(4)]

        def emit_att(j):
            for qc in range(2):
                nblk = [blk for blk in LT_BLOCKS if blk[0] == qc]
                first_kt, last_kt = nblk[0][1], nblk[-1][1]
                yps = [psy.tile([HS + 1, 512], F32, tag="psy", name="psy_e"),
                       psy.tile([HS + 1, 512], F32, tag="psy", name="psy_o")]
                for (_, kt, qs, n, off) in nblk:
                    sps = []
                    for par in range(2):           # even/odd: PE row groups
                        po = 64 * par
                        sp = psmm.tile([128, n], F32, tag="psmm", name="ps_s")
                        nc.tensor.matmul(
                            sp[:, :],
                            kt_sb[j][po:po + 64, kt * 128:(kt + 1) * 128],
                            qt_sb[j][po:po + 64, qs:qs + n],
                            start=True, stop=True,
                        )
                        sps.append(sp)
                    o = qs - qc * 512
                    for par in range(2):
                        i = 2 * j + par            # local head
                        ltb = ltpool.tile([128, n], F32, tag="lt", name="lt_t")
                        dma(ltb[:, :],
                            ltp_d[i, off:off + 128 * n].rearrange(
                                "(p m) -> p m", p=128))
                        smp = smpool.tile([128, n], BF16, tag="smp",
                                          name="smp_t")
                        nc.vector.tensor_tensor(
                            smp[:, :], sps[par][:, :], ltb[:, :],
                            op=mybir.AluOpType.is_gt,
                        )
                        nc.tensor.matmul(
                            yps[par][:, o:o + n],
                            vaug[kt][:, i * (HS + 1):(i + 1) * (HS + 1)],
                            smp[:, :],
                            start=(kt == first_kt), stop=(kt == last_kt),
                        )
                for par in range(2):
                    i = 2 * j + par
                    po = 64 * par
                    nc.scalar.copy(
                        yt_sb[j][po:po + 64, qc * 512:(qc + 1) * 512],
                        yps[par][0:64, :])
                    ast = astp.tile([65, 512], F32, tag="ast", name="ast_t")
                    nc.scalar.copy(ast[64:65, :], yps[par][64:65, :])
                    dma(asum_d[i, qc * 512:(qc + 1) * 512], ast[64:65, :])

        # interleave: qkv(j+1) emitted right after attention(j) starts
        emit_qk(0)
        for j in range(4):
            if j < 3:
                # emit next pair's qkv adjacent so Tile fills PE gaps
                emit_att(j)
                emit_qk(j + 1)
            else:
                emit_att(j)

        # ---- partial projection y_heads @ wproj_slice^T (bf16) --------
        for tt in range(8):
            ot = opool.tile([128, C], F32, tag="ot", name="ot_t")
            for nch in range(2):
                ps = psmm.tile([128, 512], F32, tag="psmm", name="ps_p")
                for jj in range(4):
                    nc.tensor.matmul(
                        ps[:, :],
                        yt_sb[jj][:, tt * 128:(tt + 1) * 128],
                        wpt_sb[jj][:, nch * 512:(nch + 1) * 512],
                        start=(jj == 0), stop=(jj == 3),
                    )
                nc.scalar.copy(ot[:, nch * 512:(nch + 1) * 512], ps[:, :])
            dma(yp_d[tt * 128:(tt + 1) * 128, :], ot[:, :])

    nc.compile()   # bacc register allocation + finalize before serialization
    return nc


def _get_nc():
    if "nc" not in _CACHE:
        _CACHE["nc"] = build_nc()
    return _CACHE["nc"]


def _get_u():
    """Exactly reproduce the uniform field jax.random.bernoulli draws.

    The container pins jax_default_prng_impl='rbg', whose bitstream is
    backend-dependent — so this must run on the same default device the
    reference uses (do NOT pin to CPU here).
    """
    import jax
    import jax.numpy as jnp
    samp_key = jax.random.fold_in(jax.random.key(0), 42)
    u = jax.random.uniform(samp_key, (B, H, T, T), dtype=jnp.float32)
    return np.asarray(u)


def _prep_ltp():
    """Pack per-core causal logit-threshold blocks: [NCORES, HPC, LT_STRIDE]."""
    if "ltp" in _CACHE:
        return _CACHE["ltp"]
    u = _get_u()
    ltp = np.empty((NCORES, HPC, LT_STRIDE), np.float32)
    kk = np.arange(128)[:, None]
    for c in range(NCORES):
        b, hg = c // 2, c % 2
        for i in range(HPC):
            h = hg * HPC + i
            for (qc, kt, qs, n, off) in LT_BLOCKS:
                ub = u[b, h, qs:qs + n, kt * 128:(kt + 1) * 128].astype(np.float64)
                with np.errstate(divide="ignore"):
                    lt = np.log(ub) - np.log1p(-ub)
                ltb = np.ascontiguousarray(lt.T).astype(np.float32)  # [128, n]
                if qs == kt * 128:
                    ltb[kk > np.arange(n)[None, :]] = np.inf
                ltp[c, i, off:off + 128 * n] = ltb.reshape(-1)
    _CACHE["ltp"] = ltp
    return ltp


def make_in_maps(x, w_attn, w_proj):
    scale = np.float32(1.0 / np.sqrt(np.float32(HS)))   # 0.125, exact pow2
    ltp = _prep_ltp()
    in_maps = []
    for c in range(NCORES):
        b, hg = c // 2, c % 2
        r0 = hg * HPC * HS
        in_maps.append({
            "xt": np.ascontiguousarray(x[b].T),
            "wqt": np.ascontiguousarray((w_attn[r0:r0 + 512, :] * scale).T),
            "wkt": np.ascontiguousarray(w_attn[C + r0:C + r0 + 512, :].T),
            "wvt": np.ascontiguousarray(
                w_attn[2 * C + r0:2 * C + r0 + 512, :].T).astype(
                    ml_dtypes.bfloat16),
            "wpt": np.ascontiguousarray(
                w_proj[:, r0:r0 + 512].T).astype(ml_dtypes.bfloat16),
            "ltp": ltp[c],
        })
    return in_maps


def assemble(results):
    """Combine per-core outputs into full outputs."""
    y = np.zeros((B, T, C), np.float32)
    att_sum = np.zeros((B, H, T), np.float32)
    for c in range(NCORES):
        b, hg = c // 2, c % 2
        y[b] += results[c]["yp"]
        att_sum[b, hg * HPC:(hg + 1) * HPC, :] = results[c]["asum"]
    att_var = np.zeros((B, H, T), np.float32)
    return y, att_sum, att_var


def kernel(x, w_attn, w_proj, **run_kwargs):
    x = np.asarray(x, dtype=np.float32)
    w_attn = np.asarray(w_attn, dtype=np.float32)
    w_proj = np.asarray(w_proj, dtype=np.float32)
    nc = _get_nc()
    in_maps = make_in_maps(x, w_attn, w_proj)
    res = run_bass_kernel_spmd(nc, in_maps, core_ids=list(range(NCORES)),
                               **run_kwargs)
    out = assemble(res.results)
    _CACHE["last_result"] = res
    return out
